# revision 46
# baseline (speedup 1.0000x reference)
"""Trainium2 Bass kernel for nn_MeshTransformer (8-core SPMD, V-sharded).

Computes, for each of BS=256 (b,s) pairs:
    out[bs, v, i] = sum_{p,j} ws[bs,p] * R[i,j](bs,p) * deformed[p,v,j]
                    + sum_p w[bs,p] * t[bs,p,i]
with R the XYZ-euler rotation, ws = w * scale, deformed = base + offsets.

Mapping:
  - Vertex dim V (2562, padded to 2576) is sharded 8 ways (322/core).
  - The einsum's contraction (p,j) [K=192] runs on the PE as 6 PSUM groups
    (3 output planes x 2 batch halves), 2 accumulated fp16 matmuls each:
      mm-a: LA_i = [ws*R_i0 (p 0..63) ; ws*R_i1 (p 64..127)]  [K=128]
            against DA = [def_0 ; def_1]
      mm-b: LBp_i = h-split ws*R_i2 (batch half h at partitions h*64..)
            against DB = [def_2 ; def_2]                      [K=64]
    and the translation term (host-reduced to [256,3]) is added as a
    per-partition bias during the PSUM drain (ACT scalar.add / DVE
    tensor_scalar_add, alternating so both engines overlap).
  - The small per-(bs,p) weight matrices (256x64x9 values) are built on
    the host and shipped ready-to-use; the device is pure DMA + PE +
    drain, so nothing gates the matmuls but the input DMA itself.
  - Input rides 3 DMAs sized so their arrival paces the PE exactly
    (i=0 weights + rhs first; in2 via Pool SWDGE to dodge the serial SP
    issue queue); a wait-queue-stuffing trick delays the matmuls' seq
    dispatch past the p-state ramp window so they run at full clock.
  - Output leaves in 3 DMAs (SP / ACT / SP) ordered by drain completion
    so the DMA-engine queue runs gapless from the first drained plane.
"""

import numpy as np
from contextlib import ExitStack

import concourse.bass as bass
import concourse.tile as tile
from concourse import bacc, mybir
from concourse.bass_utils import run_bass_kernel_spmd

B, S, P, V = 16, 16, 64, 2562
BS = B * S              # 256
N_CORES = 8
VPAD = 2576             # multiple of 8; per-core N kept even
VC = VPAD // N_CORES    # 322 vertices per core

F32 = mybir.dt.float32
F16 = mybir.dt.float16


def _build_kernel():
    nc = bacc.Bacc("TRN2", target_bir_lowering=False, debug=False)

    # in1: LA0 | LBp0 | da | db | tsb   (i=0 weights + rhs + translation
    # bias; tsb is 6 fp32 values shipped as 12 fp16 columns, bitcast on device)
    in1_d = nc.dram_tensor("in1", [128, 384 + 2 * VC + 12], F16,
                           kind="ExternalInput").ap()
    # in2: LA1 | LBp1      in3: LA2 | LBp2
    in2_d = nc.dram_tensor("in2", [128, 384], F16, kind="ExternalInput").ap()
    in3_d = nc.dram_tensor("in3", [128, 384], F16, kind="ExternalInput").ap()
    # out columns: [d00 d10 d20 | d01 d11 d21], each VC wide
    out_d = nc.dram_tensor("out", [128, 6 * VC], F16,
                           kind="ExternalOutput").ap()

    with tile.TileContext(nc) as tc, ExitStack() as ctx:
        pool = ctx.enter_context(tc.tile_pool(name="work", bufs=1))
        psum = ctx.enter_context(tc.tile_pool(name="psum", bufs=6, space="PSUM"))
        psumw = ctx.enter_context(tc.tile_pool(name="psumw", bufs=1, space="PSUM"))

        # preload the ACT function table while the inputs are in flight
        dummy = pool.tile([128, 1], F16, tag="dummy")
        dummy2 = pool.tile([128, 1], F16, tag="dummy2")
        nc.vector.memset(dummy[:], 0.25)
        nc.scalar.add(dummy2[:], dummy[:], dummy[:, 0:1])
        # PE p-state warm-up: a 1-col matmul with no data dependencies
        wps = psumw.tile([1, 1], F32)
        nc.tensor.matmul(wps[:], dummy[:], dummy[:], start=True, stop=True)

        t1 = pool.tile([128, 384 + 2 * VC + 12], F16, tag="t1")
        t2 = pool.tile([128, 384], F16, tag="t2")
        t3 = pool.tile([128, 384], F16, tag="t3")
        nc.sync.dma_start(out=t1[:], in_=in1_d[:])
        # in2 via Pool SWDGE: bypasses the serial SP issue queue and HWDGE;
        # its chain is ready first, so it takes the DMA-engine slot right
        # after in1 (arrival order sets the queue order)
        nc.gpsimd.dma_start(out=t2[:], in_=in2_d[:])
        nc.sync.dma_start(out=t3[:], in_=in3_d[:])

        # fill the PE wait queue with no-op matmuls gated on t1 so the real
        # matmuls' seq dispatch (where the p-state is sampled) happens after
        # the ramp window and they run at full clock
        for k in range(4):
            nc.tensor.matmul(wps[:], t1[:, 0:1], dummy[:],
                             start=True, stop=True)

        la = [t1[:, 0:256], t2[:, 0:256], t3[:, 0:256]]
        # LBp_i [128,128]: rows 0:64 = ws*R_i2 for bs 0:128, rows 64:128 for
        # bs 128:256 (h-split halves the LB payload); db rows are def_2 twice
        lbp = [t1[:, 256:384], t2[:, 256:384], t3[:, 256:384]]
        da = t1[:, 384:384 + VC]
        db = t1[:, 384 + VC:384 + 2 * VC]
        tsb = t1[:, 384 + 2 * VC:384 + 2 * VC + 12].bitcast(F32)

        osb = pool.tile([128, 6 * VC], F16, tag="osb")

        # h0 groups first so their drains (and the first out DMA) fire early
        order = [(0, 0), (1, 0), (2, 0), (0, 1), (1, 1), (2, 1)]
        pss = {}
        for i, h in order:
            ms = slice(h * 128, (h + 1) * 128)
            ks = slice(h * 64, (h + 1) * 64)
            ps = psum.tile([128, VC], F32)
            nc.tensor.matmul(ps[:], la[i][:, ms], da, start=True, stop=False)
            nc.tensor.matmul(ps[:], lbp[i][ks, :], db[ks, :],
                             start=False, stop=True)
            pss[(i, h)] = ps

        # drains add the translation term as a per-partition bias;
        # alternate ACT/DVE in matmul completion order
        def bias(i, h):
            return tsb[:, h * 3 + i:h * 3 + i + 1]

        nc.scalar.add(osb[:, 0:VC], pss[(0, 0)][:], bias(0, 0))
        nc.vector.tensor_scalar_add(osb[:, VC:2 * VC], pss[(1, 0)][:], bias(1, 0))
        nc.scalar.add(osb[:, 2 * VC:3 * VC], pss[(2, 0)][:], bias(2, 0))
        nc.vector.tensor_scalar_add(osb[:, 3 * VC:4 * VC], pss[(0, 1)][:],
                                    bias(0, 1))
        nc.scalar.add(osb[:, 4 * VC:5 * VC], pss[(1, 1)][:], bias(1, 1))
        nc.vector.tensor_scalar_add(osb[:, 5 * VC:6 * VC], pss[(2, 1)][:],
                                    bias(2, 1))

        # out pieces issued from three engines so the DMA queue stays gapless
        nc.sync.dma_start(out=out_d[:, 0:3 * VC], in_=osb[:, 0:3 * VC])
        nc.scalar.dma_start(out=out_d[:, 3 * VC:5 * VC], in_=osb[:, 3 * VC:5 * VC])
        nc.sync.dma_start(out=out_d[:, 5 * VC:6 * VC], in_=osb[:, 5 * VC:6 * VC])

    nc.compile()
    return nc


_NC_CACHE = None


def _get_nc():
    global _NC_CACHE
    if _NC_CACHE is None:
        _NC_CACHE = _build_kernel()
    return _NC_CACHE


def _prep_inputs(scales, transforms, prototype_weights, prototype_offsets, base_verts):
    """Host-side shard/layout prep: rotation-matrix build + packing."""
    f = np.float64
    hh = np.float16
    scl1 = np.asarray(scales, np.float32).reshape(BS).astype(f)
    tf = np.asarray(transforms, np.float32).reshape(BS, P, 6).astype(f)
    w = np.asarray(prototype_weights, np.float32).reshape(BS, P).astype(f)
    t = tf[:, :, 0:3]                       # [bs,p,3]
    sa, ca = np.sin(tf[:, :, 3]), np.cos(tf[:, :, 3])
    sb, cb = np.sin(tf[:, :, 4]), np.cos(tf[:, :, 4])
    sc, cc = np.sin(tf[:, :, 5]), np.cos(tf[:, :, 5])

    # R = Rx(a) @ Ry(b) @ Rz(c)  (pytorch3d euler 'XYZ')
    R = np.empty((BS, P, 3, 3), f)
    R[..., 0, 0] = cb * cc
    R[..., 0, 1] = -cb * sc
    R[..., 0, 2] = sb
    R[..., 1, 0] = ca * sc + sa * sb * cc
    R[..., 1, 1] = ca * cc - sa * sb * sc
    R[..., 1, 2] = -sa * cb
    R[..., 2, 0] = sa * sc - ca * sb * cc
    R[..., 2, 1] = sa * cc + ca * sb * sc
    R[..., 2, 2] = ca * cb

    Rws = R * (w * scl1[:, None])[..., None, None]   # [bs,p,i,j]
    tsum = (w[..., None] * t).sum(axis=1)            # [bs,3] translation term

    LA = np.empty((3, 128, BS), f)
    LBp = np.empty((3, 128, 128), f)
    for i in range(3):
        LA[i, 0:64] = Rws[:, :, i, 0].T
        LA[i, 64:128] = Rws[:, :, i, 1].T
        r2 = Rws[:, :, i, 2].T                       # [p=64, bs=256]
        LBp[i, 0:64] = r2[:, 0:128]                  # bs half 0
        LBp[i, 64:128] = r2[:, 128:256]              # bs half 1

    tsb = np.empty((128, 6), np.float32)
    for h in range(2):
        for i in range(3):
            tsb[:, h * 3 + i] = tsum[h * 128:(h + 1) * 128, i]
    tsb = np.ascontiguousarray(tsb).view(np.float16)         # [128, 12]

    in2 = np.concatenate([LA[1], LBp[1]], axis=1).astype(hh)  # [128, 384]
    in3 = np.concatenate([LA[2], LBp[2]], axis=1).astype(hh)  # [128, 384]

    deff = np.zeros((P, VPAD, 3), np.float32)
    deff[:, :V] = (np.asarray(base_verts, np.float32)[None]
                   + np.asarray(prototype_offsets, np.float32))

    lw0 = np.concatenate([LA[0], LBp[0]], axis=1)             # [128, 384]
    in_maps = []
    for core in range(N_CORES):
        vs = slice(core * VC, (core + 1) * VC)
        dab = np.empty((128, 2 * VC), np.float32)
        dab[0:64, 0:VC] = deff[:, vs, 0]
        dab[64:128, 0:VC] = deff[:, vs, 1]
        dab[0:64, VC:2 * VC] = deff[:, vs, 2]
        dab[64:128, VC:2 * VC] = deff[:, vs, 2]      # def_2 again for h=1
        in1 = np.concatenate(
            [np.concatenate([lw0, dab], axis=1).astype(hh), tsb], axis=1)
        in_maps.append({"in1": in1, "in2": in2, "in3": in3})
    return in_maps


def kernel(scales, transforms, prototype_weights, prototype_offsets, base_verts):
    nc = _get_nc()
    in_maps = _prep_inputs(
        scales, transforms, prototype_weights, prototype_offsets, base_verts)
    res = run_bass_kernel_spmd(nc, in_maps, list(range(N_CORES)))
    full = np.empty((BS, VPAD, 3), np.float32)
    for c in range(N_CORES):
        planes = res.results[c]["out"].astype(np.float32)  # [128, 6*VC]
        vs = slice(c * VC, (c + 1) * VC)
        for h in range(2):
            for i in range(3):
                col = (h * 3 + i) * VC
                full[h * 128:(h + 1) * 128, vs, i] = \
                    planes[:, col:col + VC]
    return np.ascontiguousarray(full[:, :V, :])


# revision 47
# speedup vs baseline: 1.0036x; 1.0036x over previous
"""Trainium2 Bass kernel for nn_MeshTransformer (8-core SPMD, V-sharded).

Computes, for each of BS=256 (b,s) pairs:
    out[bs, v, i] = sum_{p,j} ws[bs,p] * R[i,j](bs,p) * deformed[p,v,j]
                    + sum_p w[bs,p] * t[bs,p,i]
with R the XYZ-euler rotation, ws = w * scale, deformed = base + offsets.

Mapping:
  - Vertex dim V (2562, padded to 2576) is sharded 8 ways (322/core).
  - The einsum's contraction (p,j) [K=192] runs on the PE as 6 PSUM groups
    (3 output planes x 2 batch halves), 2 accumulated fp16 matmuls each:
      mm-a: LA_i = [ws*R_i0 (p 0..63) ; ws*R_i1 (p 64..127)]  [K=128]
            against DA = [def_0 ; def_1]
      mm-b: LBp_i = h-split ws*R_i2 (batch half h at partitions h*64..)
            against DB = [def_2 ; def_2]                      [K=64]
    and the translation term (host-reduced to [256,3]) is added as a
    per-partition bias during the PSUM drain (ACT scalar.add / DVE
    tensor_scalar_add, alternating so both engines overlap).
  - The small per-(bs,p) weight matrices (256x64x9 values) are built on
    the host and shipped ready-to-use; the device is pure DMA + PE +
    drain, so nothing gates the matmuls but the input DMA itself.
  - Input rides 3 DMAs sized so their arrival paces the PE exactly
    (i=0 weights + rhs first; in2 via Pool SWDGE to dodge the serial SP
    issue queue); a wait-queue-stuffing trick delays the matmuls' seq
    dispatch past the p-state ramp window so they run at full clock.
  - Output leaves in 3 DMAs (SP / ACT / SP) ordered by drain completion
    so the DMA-engine queue runs gapless from the first drained plane.
"""

import numpy as np
from contextlib import ExitStack

import concourse.bass as bass
import concourse.tile as tile
from concourse import bacc, mybir
from concourse.bass_utils import run_bass_kernel_spmd

B, S, P, V = 16, 16, 64, 2562
BS = B * S              # 256
N_CORES = 8
VPAD = 2576             # multiple of 8; per-core N kept even
VC = VPAD // N_CORES    # 322 vertices per core

F32 = mybir.dt.float32
F16 = mybir.dt.float16


def _build_kernel():
    nc = bacc.Bacc("TRN2", target_bir_lowering=False, debug=False)

    # in1: LA0 | LBp0 | da | db | tsb   (i=0 weights + rhs + translation
    # bias; tsb is 6 fp32 values shipped as 12 fp16 columns, bitcast on device)
    in1_d = nc.dram_tensor("in1", [128, 384 + 2 * VC + 12], F16,
                           kind="ExternalInput").ap()
    # in2: LA1 | LBp1      in3: LA2 | LBp2
    in2_d = nc.dram_tensor("in2", [128, 384], F16, kind="ExternalInput").ap()
    in3_d = nc.dram_tensor("in3", [128, 384], F16, kind="ExternalInput").ap()
    # out columns: [d00 d10 d20 | d01 d11 d21], each VC wide
    out_d = nc.dram_tensor("out", [128, 6 * VC], F16,
                           kind="ExternalOutput").ap()

    with tile.TileContext(nc) as tc, ExitStack() as ctx:
        pool = ctx.enter_context(tc.tile_pool(name="work", bufs=1))
        psum = ctx.enter_context(tc.tile_pool(name="psum", bufs=6, space="PSUM"))
        psumw = ctx.enter_context(tc.tile_pool(name="psumw", bufs=1, space="PSUM"))

        # preload the ACT function table while the inputs are in flight
        dummy = pool.tile([128, 1], F16, tag="dummy")
        dummy2 = pool.tile([128, 1], F16, tag="dummy2")
        nc.vector.memset(dummy[:], 0.25)
        nc.scalar.add(dummy2[:], dummy[:], dummy[:, 0:1])
        # PE p-state warm-up: a 1-col matmul with no data dependencies
        wps = psumw.tile([1, 1], F32)
        nc.tensor.matmul(wps[:], dummy[:], dummy[:], start=True, stop=True)

        t1 = pool.tile([128, 384 + 2 * VC + 12], F16, tag="t1")
        t2 = pool.tile([128, 384], F16, tag="t2")
        t3 = pool.tile([128, 384], F16, tag="t3")
        nc.sync.dma_start(out=t1[:], in_=in1_d[:])
        # in2 via Pool SWDGE: bypasses the serial SP issue queue and HWDGE;
        # its chain is ready first, so it takes the DMA-engine slot right
        # after in1 (arrival order sets the queue order)
        nc.gpsimd.dma_start(out=t2[:], in_=in2_d[:])
        nc.sync.dma_start(out=t3[:], in_=in3_d[:])

        # fill the PE wait queue with no-op matmuls gated on t1 so the real
        # matmuls' seq dispatch (where the p-state is sampled) happens after
        # the ramp window and they run at full clock
        for k in range(4):
            nc.tensor.matmul(wps[:], t1[:, 0:1], dummy[:],
                             start=True, stop=True)

        la = [t1[:, 0:256], t2[:, 0:256], t3[:, 0:256]]
        # LBp_i [128,128]: rows 0:64 = ws*R_i2 for bs 0:128, rows 64:128 for
        # bs 128:256 (h-split halves the LB payload); db rows are def_2 twice
        lbp = [t1[:, 256:384], t2[:, 256:384], t3[:, 256:384]]
        da = t1[:, 384:384 + VC]
        db = t1[:, 384 + VC:384 + 2 * VC]
        tsb = t1[:, 384 + 2 * VC:384 + 2 * VC + 12].bitcast(F32)

        osb = pool.tile([128, 6 * VC], F16, tag="osb")

        # h0 groups first so their drains (and the first out DMA) fire early
        order = [(0, 0), (1, 0), (2, 0), (0, 1), (1, 1), (2, 1)]
        pss = {}
        for i, h in order:
            ms = slice(h * 128, (h + 1) * 128)
            ks = slice(h * 64, (h + 1) * 64)
            ps = psum.tile([128, VC], F32)
            nc.tensor.matmul(ps[:], la[i][:, ms], da, start=True, stop=False)
            nc.tensor.matmul(ps[:], lbp[i][ks, :], db[ks, :],
                             start=False, stop=True)
            pss[(i, h)] = ps

        # drains add the translation term as a per-partition bias;
        # alternate ACT/DVE in matmul completion order
        def bias(i, h):
            return tsb[:, h * 3 + i:h * 3 + i + 1]

        nc.scalar.add(osb[:, 0:VC], pss[(0, 0)][:], bias(0, 0))
        nc.vector.tensor_scalar_add(osb[:, VC:2 * VC], pss[(1, 0)][:], bias(1, 0))
        nc.scalar.add(osb[:, 2 * VC:3 * VC], pss[(2, 0)][:], bias(2, 0))
        nc.vector.tensor_scalar_add(osb[:, 3 * VC:4 * VC], pss[(0, 1)][:],
                                    bias(0, 1))
        nc.scalar.add(osb[:, 4 * VC:5 * VC], pss[(1, 1)][:], bias(1, 1))
        nc.vector.tensor_scalar_add(osb[:, 5 * VC:6 * VC], pss[(2, 1)][:],
                                    bias(2, 1))

        # out pieces all on SP, split so each issue's HWDGE gen runs inside
        # its own SEQ window (no gen queuing) and the queue stays gapless
        nc.sync.dma_start(out=out_d[:, 0:2 * VC], in_=osb[:, 0:2 * VC])
        nc.sync.dma_start(out=out_d[:, 2 * VC:5 * VC], in_=osb[:, 2 * VC:5 * VC])
        nc.sync.dma_start(out=out_d[:, 5 * VC:6 * VC], in_=osb[:, 5 * VC:6 * VC])

    nc.compile()
    return nc


_NC_CACHE = None


def _get_nc():
    global _NC_CACHE
    if _NC_CACHE is None:
        _NC_CACHE = _build_kernel()
    return _NC_CACHE


def _prep_inputs(scales, transforms, prototype_weights, prototype_offsets, base_verts):
    """Host-side shard/layout prep: rotation-matrix build + packing."""
    f = np.float64
    hh = np.float16
    scl1 = np.asarray(scales, np.float32).reshape(BS).astype(f)
    tf = np.asarray(transforms, np.float32).reshape(BS, P, 6).astype(f)
    w = np.asarray(prototype_weights, np.float32).reshape(BS, P).astype(f)
    t = tf[:, :, 0:3]                       # [bs,p,3]
    sa, ca = np.sin(tf[:, :, 3]), np.cos(tf[:, :, 3])
    sb, cb = np.sin(tf[:, :, 4]), np.cos(tf[:, :, 4])
    sc, cc = np.sin(tf[:, :, 5]), np.cos(tf[:, :, 5])

    # R = Rx(a) @ Ry(b) @ Rz(c)  (pytorch3d euler 'XYZ')
    R = np.empty((BS, P, 3, 3), f)
    R[..., 0, 0] = cb * cc
    R[..., 0, 1] = -cb * sc
    R[..., 0, 2] = sb
    R[..., 1, 0] = ca * sc + sa * sb * cc
    R[..., 1, 1] = ca * cc - sa * sb * sc
    R[..., 1, 2] = -sa * cb
    R[..., 2, 0] = sa * sc - ca * sb * cc
    R[..., 2, 1] = sa * cc + ca * sb * sc
    R[..., 2, 2] = ca * cb

    Rws = R * (w * scl1[:, None])[..., None, None]   # [bs,p,i,j]
    tsum = (w[..., None] * t).sum(axis=1)            # [bs,3] translation term

    LA = np.empty((3, 128, BS), f)
    LBp = np.empty((3, 128, 128), f)
    for i in range(3):
        LA[i, 0:64] = Rws[:, :, i, 0].T
        LA[i, 64:128] = Rws[:, :, i, 1].T
        r2 = Rws[:, :, i, 2].T                       # [p=64, bs=256]
        LBp[i, 0:64] = r2[:, 0:128]                  # bs half 0
        LBp[i, 64:128] = r2[:, 128:256]              # bs half 1

    tsb = np.empty((128, 6), np.float32)
    for h in range(2):
        for i in range(3):
            tsb[:, h * 3 + i] = tsum[h * 128:(h + 1) * 128, i]
    tsb = np.ascontiguousarray(tsb).view(np.float16)         # [128, 12]

    in2 = np.concatenate([LA[1], LBp[1]], axis=1).astype(hh)  # [128, 384]
    in3 = np.concatenate([LA[2], LBp[2]], axis=1).astype(hh)  # [128, 384]

    deff = np.zeros((P, VPAD, 3), np.float32)
    deff[:, :V] = (np.asarray(base_verts, np.float32)[None]
                   + np.asarray(prototype_offsets, np.float32))

    lw0 = np.concatenate([LA[0], LBp[0]], axis=1)             # [128, 384]
    in_maps = []
    for core in range(N_CORES):
        vs = slice(core * VC, (core + 1) * VC)
        dab = np.empty((128, 2 * VC), np.float32)
        dab[0:64, 0:VC] = deff[:, vs, 0]
        dab[64:128, 0:VC] = deff[:, vs, 1]
        dab[0:64, VC:2 * VC] = deff[:, vs, 2]
        dab[64:128, VC:2 * VC] = deff[:, vs, 2]      # def_2 again for h=1
        in1 = np.concatenate(
            [np.concatenate([lw0, dab], axis=1).astype(hh), tsb], axis=1)
        in_maps.append({"in1": in1, "in2": in2, "in3": in3})
    return in_maps


def kernel(scales, transforms, prototype_weights, prototype_offsets, base_verts):
    nc = _get_nc()
    in_maps = _prep_inputs(
        scales, transforms, prototype_weights, prototype_offsets, base_verts)
    res = run_bass_kernel_spmd(nc, in_maps, list(range(N_CORES)))
    full = np.empty((BS, VPAD, 3), np.float32)
    for c in range(N_CORES):
        planes = res.results[c]["out"].astype(np.float32)  # [128, 6*VC]
        vs = slice(c * VC, (c + 1) * VC)
        for h in range(2):
            for i in range(3):
                col = (h * 3 + i) * VC
                full[h * 128:(h + 1) * 128, vs, i] = \
                    planes[:, col:col + VC]
    return np.ascontiguousarray(full[:, :V, :])


# revision 48
# speedup vs baseline: 1.0066x; 1.0030x over previous
"""Trainium2 Bass kernel for nn_MeshTransformer (8-core SPMD, V-sharded).

Computes, for each of BS=256 (b,s) pairs:
    out[bs, v, i] = sum_{p,j} ws[bs,p] * R[i,j](bs,p) * deformed[p,v,j]
                    + sum_p w[bs,p] * t[bs,p,i]
with R the XYZ-euler rotation, ws = w * scale, deformed = base + offsets.

Mapping:
  - Vertex dim V (2562, padded to 2576) is sharded 8 ways (322/core).
  - The einsum's contraction (p,j) [K=192] runs on the PE as 6 PSUM groups
    (3 output planes x 2 batch halves), 2 accumulated fp16 matmuls each:
      mm-a: LA_i = [ws*R_i0 (p 0..63) ; ws*R_i1 (p 64..127)]  [K=128]
            against DA = [def_0 ; def_1]
      mm-b: LBp_i = h-split ws*R_i2 (batch half h at partitions h*64..)
            against DB = [def_2 ; def_2]                      [K=64]
    and the translation term (host-reduced to [256,3]) is added as a
    per-partition bias during the PSUM drain (ACT scalar.add / DVE
    tensor_scalar_add, alternating so both engines overlap).
  - The small per-(bs,p) weight matrices (256x64x9 values) are built on
    the host and shipped ready-to-use; the device is pure DMA + PE +
    drain, so nothing gates the matmuls but the input DMA itself.
  - Input rides 3 DMAs sized so their arrival paces the PE exactly
    (i=0 weights + rhs first; in2 via Pool SWDGE to dodge the serial SP
    issue queue); a wait-queue-stuffing trick delays the matmuls' seq
    dispatch past the p-state ramp window so they run at full clock.
  - Output leaves in 3 DMAs (SP / ACT / SP) ordered by drain completion
    so the DMA-engine queue runs gapless from the first drained plane.
"""

import numpy as np
from contextlib import ExitStack

import concourse.bass as bass
import concourse.tile as tile
from concourse import bacc, mybir
from concourse.bass_utils import run_bass_kernel_spmd

B, S, P, V = 16, 16, 64, 2562
BS = B * S              # 256
N_CORES = 8
VPAD = 2576             # multiple of 8; per-core N kept even
VC = VPAD // N_CORES    # 322 vertices per core

F32 = mybir.dt.float32
F16 = mybir.dt.float16


def _build_kernel():
    nc = bacc.Bacc("TRN2", target_bir_lowering=False, debug=False)

    # in1: LA0 | LBp0 | da | db | tsb   (i=0 weights + rhs + translation
    # bias; tsb is 6 fp32 values shipped as 12 fp16 columns, bitcast on device)
    in1_d = nc.dram_tensor("in1", [128, 384 + 2 * VC + 12], F16,
                           kind="ExternalInput").ap()
    # in2: LA1 | LBp1      in3: LA2 | LBp2
    in2_d = nc.dram_tensor("in2", [128, 384], F16, kind="ExternalInput").ap()
    in3_d = nc.dram_tensor("in3", [128, 384], F16, kind="ExternalInput").ap()
    # out columns: [d00 d10 d20 | d01 d11 d21], each VC wide
    out_d = nc.dram_tensor("out", [128, 6 * VC], F16,
                           kind="ExternalOutput").ap()

    with tile.TileContext(nc) as tc, ExitStack() as ctx:
        pool = ctx.enter_context(tc.tile_pool(name="work", bufs=1))
        psum = ctx.enter_context(tc.tile_pool(name="psum", bufs=6, space="PSUM"))
        psumw = ctx.enter_context(tc.tile_pool(name="psumw", bufs=1, space="PSUM"))

        # preload the ACT function table while the inputs are in flight
        dummy = pool.tile([128, 1], F16, tag="dummy")
        dummy2 = pool.tile([128, 1], F16, tag="dummy2")
        nc.vector.memset(dummy[:], 0.25)
        nc.scalar.add(dummy2[:], dummy[:], dummy[:, 0:1])
        # PE p-state warm-up: a 1-col matmul with no data dependencies
        wps = psumw.tile([1, 1], F32)
        nc.tensor.matmul(wps[:], dummy[:], dummy[:], start=True, stop=True)

        t1 = pool.tile([128, 384 + 2 * VC + 12], F16, tag="t1")
        t2 = pool.tile([128, 384], F16, tag="t2")
        t3 = pool.tile([128, 384], F16, tag="t3")
        nc.sync.dma_start(out=t1[:], in_=in1_d[:])
        # in2 via Pool SWDGE: bypasses the serial SP issue queue and HWDGE;
        # its chain is ready first, so it takes the DMA-engine slot right
        # after in1 (arrival order sets the queue order)
        nc.gpsimd.dma_start(out=t2[:], in_=in2_d[:])
        nc.sync.dma_start(out=t3[:], in_=in3_d[:])

        # fill the PE wait queue with no-op matmuls gated on t1 so the real
        # matmuls' seq dispatch (where the p-state is sampled) happens after
        # the ramp window and they run at full clock
        for k in range(4):
            nc.tensor.matmul(wps[:], t1[:, 0:1], dummy[:],
                             start=True, stop=True)

        la = [t1[:, 0:256], t2[:, 0:256], t3[:, 0:256]]
        # LBp_i [128,128]: rows 0:64 = ws*R_i2 for bs 0:128, rows 64:128 for
        # bs 128:256 (h-split halves the LB payload); db rows are def_2 twice
        lbp = [t1[:, 256:384], t2[:, 256:384], t3[:, 256:384]]
        da = t1[:, 384:384 + VC]
        db = t1[:, 384 + VC:384 + 2 * VC]
        tsb = t1[:, 384 + 2 * VC:384 + 2 * VC + 12].bitcast(F32)

        osb = pool.tile([128, 6 * VC], F16, tag="osb")

        # h0 groups first so their drains (and the first out DMA) fire early
        order = [(0, 0), (1, 0), (2, 0), (0, 1), (1, 1), (2, 1)]
        pss = {}
        for i, h in order:
            ms = slice(h * 128, (h + 1) * 128)
            ks = slice(h * 64, (h + 1) * 64)
            ps = psum.tile([128, VC], F32)
            nc.tensor.matmul(ps[:], la[i][:, ms], da, start=True, stop=False)
            nc.tensor.matmul(ps[:], lbp[i][ks, :], db[ks, :],
                             start=False, stop=True)
            pss[(i, h)] = ps

        # drains add the translation term as a per-partition bias;
        # alternate ACT/DVE in matmul completion order
        def bias(i, h):
            return tsb[:, h * 3 + i:h * 3 + i + 1]

        nc.scalar.add(osb[:, 0:VC], pss[(0, 0)][:], bias(0, 0))
        nc.vector.tensor_scalar_add(osb[:, VC:2 * VC], pss[(1, 0)][:], bias(1, 0))
        nc.scalar.add(osb[:, 2 * VC:3 * VC], pss[(2, 0)][:], bias(2, 0))
        nc.vector.tensor_scalar_add(osb[:, 3 * VC:4 * VC], pss[(0, 1)][:],
                                    bias(0, 1))
        nc.scalar.add(osb[:, 4 * VC:5 * VC], pss[(1, 1)][:], bias(1, 1))
        nc.vector.tensor_scalar_add(osb[:, 5 * VC:6 * VC], pss[(2, 1)][:],
                                    bias(2, 1))

        # out pieces all on SP, split so each issue's HWDGE gen runs inside
        # its own SEQ window (no gen queuing) and the queue stays gapless
        # (split points found by sweeping TimelineSim over all partitions)
        nc.sync.dma_start(out=out_d[:, 0:VC], in_=osb[:, 0:VC])
        nc.sync.dma_start(out=out_d[:, VC:4 * VC], in_=osb[:, VC:4 * VC])
        nc.sync.dma_start(out=out_d[:, 4 * VC:6 * VC], in_=osb[:, 4 * VC:6 * VC])

    nc.compile()
    return nc


_NC_CACHE = None


def _get_nc():
    global _NC_CACHE
    if _NC_CACHE is None:
        _NC_CACHE = _build_kernel()
    return _NC_CACHE


def _prep_inputs(scales, transforms, prototype_weights, prototype_offsets, base_verts):
    """Host-side shard/layout prep: rotation-matrix build + packing."""
    f = np.float64
    hh = np.float16
    scl1 = np.asarray(scales, np.float32).reshape(BS).astype(f)
    tf = np.asarray(transforms, np.float32).reshape(BS, P, 6).astype(f)
    w = np.asarray(prototype_weights, np.float32).reshape(BS, P).astype(f)
    t = tf[:, :, 0:3]                       # [bs,p,3]
    sa, ca = np.sin(tf[:, :, 3]), np.cos(tf[:, :, 3])
    sb, cb = np.sin(tf[:, :, 4]), np.cos(tf[:, :, 4])
    sc, cc = np.sin(tf[:, :, 5]), np.cos(tf[:, :, 5])

    # R = Rx(a) @ Ry(b) @ Rz(c)  (pytorch3d euler 'XYZ')
    R = np.empty((BS, P, 3, 3), f)
    R[..., 0, 0] = cb * cc
    R[..., 0, 1] = -cb * sc
    R[..., 0, 2] = sb
    R[..., 1, 0] = ca * sc + sa * sb * cc
    R[..., 1, 1] = ca * cc - sa * sb * sc
    R[..., 1, 2] = -sa * cb
    R[..., 2, 0] = sa * sc - ca * sb * cc
    R[..., 2, 1] = sa * cc + ca * sb * sc
    R[..., 2, 2] = ca * cb

    Rws = R * (w * scl1[:, None])[..., None, None]   # [bs,p,i,j]
    tsum = (w[..., None] * t).sum(axis=1)            # [bs,3] translation term

    LA = np.empty((3, 128, BS), f)
    LBp = np.empty((3, 128, 128), f)
    for i in range(3):
        LA[i, 0:64] = Rws[:, :, i, 0].T
        LA[i, 64:128] = Rws[:, :, i, 1].T
        r2 = Rws[:, :, i, 2].T                       # [p=64, bs=256]
        LBp[i, 0:64] = r2[:, 0:128]                  # bs half 0
        LBp[i, 64:128] = r2[:, 128:256]              # bs half 1

    tsb = np.empty((128, 6), np.float32)
    for h in range(2):
        for i in range(3):
            tsb[:, h * 3 + i] = tsum[h * 128:(h + 1) * 128, i]
    tsb = np.ascontiguousarray(tsb).view(np.float16)         # [128, 12]

    in2 = np.concatenate([LA[1], LBp[1]], axis=1).astype(hh)  # [128, 384]
    in3 = np.concatenate([LA[2], LBp[2]], axis=1).astype(hh)  # [128, 384]

    deff = np.zeros((P, VPAD, 3), np.float32)
    deff[:, :V] = (np.asarray(base_verts, np.float32)[None]
                   + np.asarray(prototype_offsets, np.float32))

    lw0 = np.concatenate([LA[0], LBp[0]], axis=1)             # [128, 384]
    in_maps = []
    for core in range(N_CORES):
        vs = slice(core * VC, (core + 1) * VC)
        dab = np.empty((128, 2 * VC), np.float32)
        dab[0:64, 0:VC] = deff[:, vs, 0]
        dab[64:128, 0:VC] = deff[:, vs, 1]
        dab[0:64, VC:2 * VC] = deff[:, vs, 2]
        dab[64:128, VC:2 * VC] = deff[:, vs, 2]      # def_2 again for h=1
        in1 = np.concatenate(
            [np.concatenate([lw0, dab], axis=1).astype(hh), tsb], axis=1)
        in_maps.append({"in1": in1, "in2": in2, "in3": in3})
    return in_maps


def kernel(scales, transforms, prototype_weights, prototype_offsets, base_verts):
    nc = _get_nc()
    in_maps = _prep_inputs(
        scales, transforms, prototype_weights, prototype_offsets, base_verts)
    res = run_bass_kernel_spmd(nc, in_maps, list(range(N_CORES)))
    full = np.empty((BS, VPAD, 3), np.float32)
    for c in range(N_CORES):
        planes = res.results[c]["out"].astype(np.float32)  # [128, 6*VC]
        vs = slice(c * VC, (c + 1) * VC)
        for h in range(2):
            for i in range(3):
                col = (h * 3 + i) * VC
                full[h * 128:(h + 1) * 128, vs, i] = \
                    planes[:, col:col + VC]
    return np.ascontiguousarray(full[:, :V, :])
